# revision 1
# baseline (speedup 1.0000x reference)
"""GTU (gated Toeplitz unit) Bass kernel for 8 TRN2 NeuronCores.

Sharding: tensor-parallel over heads (H=8 -> 1 head/core). Each core
computes its head's u/v projections, the RPE-MLP Toeplitz coefficients,
the causal depthwise long-conv via dense real-DFT matmuls (circular conv
of length 2n realized as TensorE matmuls with constant DFT matrices),
the gate, and a partial o-projection. Host sums the 8 partials + o_b.
"""

import numpy as np

B, N, E = 4, 2048, 1024
H = 8
D1 = 3 * E
DH = D1 // H            # 384
R = 512
GAMMA = 0.99
EPS = 1e-8
M2 = 2 * N              # 4096 (circular conv length)
KH = M2 // 2 + 1        # 2049 rfft bins
KP = 2176               # bins padded to 17*128
KA = 1024 + 128         # augmented contraction for x (bias row), 9*128
ROWS = B * N            # 8192

_CACHE = {}


def _t3(a):
    """(M, N) -> (128, M/128, N) partition-tiled layout."""
    m, n = a.shape
    assert m % 128 == 0
    return np.ascontiguousarray(
        a.reshape(m // 128, 128, n).transpose(1, 0, 2)).astype(np.float32)


def _from3(a):
    p, m, n = a.shape
    return np.ascontiguousarray(a.transpose(1, 0, 2)).reshape(m * 128, n)


def _consts():
    if "dft" in _CACHE:
        return _CACHE["dft"]
    l = np.arange(N, dtype=np.float64)[:, None]
    k = np.arange(KP, dtype=np.float64)[None, :]
    mask = (k < KH).astype(np.float64)
    ang = 2.0 * np.pi * l * k / M2
    cr = np.cos(ang) * mask
    ci = -np.sin(ang) * mask
    dft_cri = np.concatenate([cr, ci], axis=1)            # (2048, 4352)
    w = np.where((k[0] == 0) | (k[0] == M2 // 2), 1.0, 2.0) * mask[0]
    kk = np.arange(KP, dtype=np.float64)[:, None]
    t = np.arange(N, dtype=np.float64)[None, :]
    ang2 = 2.0 * np.pi * kk * t / M2
    icos = (w[:, None] / M2) * np.cos(ang2)               # (2176, 2048)
    isin = (-w[:, None] / M2) * np.sin(ang2)
    idft_cs = np.concatenate([icos, isin], axis=0)        # (4352, 2048)
    decay = GAMMA ** np.arange(N, dtype=np.float64)       # lag 0 -> 1.0
    decay_t = decay.reshape(N // 128, 128).T              # (128, 16)
    _CACHE["dft"] = (_t3(dft_cri), _t3(idft_cs), decay_t.astype(np.float32))
    return _CACHE["dft"]


def _build():
    import concourse.bass as bass
    import concourse.mybir as mybir
    import concourse.tile as tile
    from concourse import bacc
    from concourse.kernels.tile_matmul import matmul_tile_kernel

    AFT = mybir.ActivationFunctionType
    ALU = mybir.AluOpType
    dt = mybir.dt.float32

    nc = bacc.Bacc(None, target_bir_lowering=False, debug=False, num_devices=8)

    def din(name, shape):
        return nc.dram_tensor(name, list(shape), dt, kind="ExternalInput")

    def dint(name, shape):
        return nc.dram_tensor(name, list(shape), dt)

    xTa = din("xTa", (128, KA // 128, ROWS))
    u_wa = din("u_wa", (128, KA // 128, DH))
    v_wa = din("v_wa", (128, KA // 128, DH))
    o_w = din("o_w", (128, DH // 128, E))
    p_aug = din("p_aug", (2, N))
    pw_aug = din("pw_aug", (2, R))
    lws = [din(f"lw{i}", (128, R // 128, R)) for i in range(3)]
    lbs = din("lbs", (128, 3 * (R // 128)))   # 3 layers x (128, 4)
    out_w = din("out_w", (128, R // 128, DH))
    outb = din("outb", (1, DH))
    decay = din("decay", (128, N // 128))
    dft = din("dft", (128, N // 128, 2 * KP))
    idft = din("idft", (128, 2 * KP // 128, N))
    out = nc.dram_tensor("out", [128, ROWS // 128, E], dt, kind="ExternalOutput")

    acoef = dint("acoef", (128, N // 128, DH))
    arai = dint("arai", (128, 2 * KP // 128, DH))
    xrxi = dint("xrxi", (128, B * 2 * KP // 128, DH))
    prpi = dint("prpi", (128, B * 2 * KP // 128, DH))
    uT = dint("uT", (128, DH // 128, ROWS))
    v = dint("v", (128, ROWS // 128, DH))
    tvT = dint("tvT", (128, DH // 128, ROWS))
    gT = dint("gT", (128, DH // 128, ROWS))

    KG = KP // 128            # 17 freq groups
    FG = R // 128             # 4 feature groups

    def silu_evict(nc_, psum, sbuf):
        nc_.scalar.activation(sbuf, psum, AFT.Silu)

    with tile.TileContext(nc) as tc:
        # ---------------- RPE MLP (feature-major, fully in SBUF) --------
        with (tc.tile_pool(name="mlp", bufs=1) as mp,
              tc.tile_pool(name="mlp_ps", bufs=2, space="PSUM") as mps):
            ones_col = mp.tile([128, 1], dt)      # K=128 -> M=1 reducer
            nc.vector.memset(ones_col[:], 1.0)
            one_row = mp.tile([1, 128], dt)       # K=1 -> 128-partition bcast
            nc.vector.memset(one_row[:], 1.0)
            c_sc = mp.tile([1, 1], dt)
            nc.vector.memset(c_sc[:], float(R ** -0.5))
            eps_sc = mp.tile([1, 1], dt)
            nc.vector.memset(eps_sc[:], EPS)

            pa_sb = mp.tile([2, N], dt)
            pw_sb = mp.tile([2, R], dt)
            lb_sb = mp.tile([128, 3 * FG], dt)
            nc.sync.dma_start(pa_sb[:], p_aug[:])
            nc.sync.dma_start(pw_sb[:], pw_aug[:])
            nc.sync.dma_start(lb_sb[:], lbs[:])

            h = [mp.tile([128, N], dt, name=f"h{g}", tag=f"h{g}") for g in range(FG)]
            # h0 = pos_idx @ pos_w + pos_b   (K=2), feature-major (512, 2048)
            for g in range(FG):
                for nch in range(N // 512):
                    ps = mps.tile([128, 512], dt, name="mmps", tag="mm")
                    nc.tensor.matmul(
                        ps[:], pw_sb[:, g * 128:(g + 1) * 128],
                        pa_sb[:, nch * 512:(nch + 1) * 512],
                        start=True, stop=True)
                    nc.vector.tensor_copy(h[g][:, nch * 512:(nch + 1) * 512], ps[:])

            def srms_relu(h_in, phi_out):
                # s[t] = sum_f h^2 ; factor = 1/(sqrt(s)/sqrt(R) + eps)
                sq = [mp.tile([128, N], dt, name=f"sq{g}", tag=f"sq{g}") for g in range(FG)]
                for g in range(FG):
                    nc.vector.tensor_mul(sq[g][:], h_in[g][:], h_in[g][:])
                fac = mp.tile([1, N], dt, name="fac", tag="fac")
                for nch in range(N // 512):
                    ps1 = mps.tile([1, 512], dt, name="redps", tag="red")
                    for g in range(FG):
                        nc.tensor.matmul(
                            ps1[:], ones_col[:],
                            sq[g][:, nch * 512:(nch + 1) * 512],
                            start=(g == 0), stop=(g == FG - 1))
                    sl = fac[:, nch * 512:(nch + 1) * 512]
                    nc.scalar.activation(sl, ps1[:], AFT.Sqrt)
                    nc.vector.tensor_scalar(
                        sl, sl, c_sc[:], eps_sc[:], ALU.mult, ALU.add)
                    nc.vector.reciprocal(sl, sl)
                fb = mp.tile([128, N], dt, name="fb", tag="fb")
                for nch in range(N // 512):
                    psb = mps.tile([128, 512], dt, name="bcps", tag="bc")
                    nc.tensor.matmul(
                        psb[:], one_row[:], fac[:, nch * 512:(nch + 1) * 512],
                        start=True, stop=True)
                    nc.vector.tensor_copy(fb[:, nch * 512:(nch + 1) * 512], psb[:])
                for g in range(FG):
                    nc.vector.tensor_mul(phi_out[g][:], h_in[g][:], fb[:])
                    nc.scalar.activation(phi_out[g][:], phi_out[g][:], AFT.Relu)

            phi = [mp.tile([128, N], dt, name=f"phi{g}", tag=f"phi{g}") for g in range(FG)]
            srms_relu(h, phi)

            lw_sb = mp.tile([128, FG, R], dt)
            for li in range(3):
                nc.sync.dma_start(lw_sb[:], lws[li][:])
                for g in range(FG):
                    for nch in range(N // 512):
                        ps = mps.tile([128, 512], dt, name="mmps", tag="mm")
                        for k in range(FG):
                            nc.tensor.matmul(
                                ps[:], lw_sb[:, k, g * 128:(g + 1) * 128],
                                phi[k][:, nch * 512:(nch + 1) * 512],
                                start=(k == 0), stop=(k == FG - 1))
                        sl = h[g][:, nch * 512:(nch + 1) * 512]
                        nc.vector.tensor_scalar(
                            sl, ps[:], lb_sb[:, li * FG + g:li * FG + g + 1],
                            None, ALU.add)
                srms_relu(h, phi)

            # coefs (t-major) = phi.T @ out_w  -> * decay + out_b -> acoef
            ow_sb = mp.tile([128, FG, DH], dt)
            ob_sb = mp.tile([1, DH], dt)
            dec_sb = mp.tile([128, N // 128], dt)
            nc.sync.dma_start(ow_sb[:], out_w[:])
            nc.sync.dma_start(ob_sb[:], outb[:])
            nc.sync.dma_start(dec_sb[:], decay[:])
            obb = mp.tile([128, DH], dt)
            psb = mps.tile([128, DH], dt, name="bc2ps", tag="bc")
            nc.tensor.matmul(psb[:], one_row[:], ob_sb[:], start=True, stop=True)
            nc.vector.tensor_copy(obb[:], psb[:])
            for m in range(N // 128):
                ps = mps.tile([128, DH], dt, name="mm2ps", tag="mm")
                for k in range(FG):
                    nc.tensor.matmul(
                        ps[:], phi[k][:, m * 128:(m + 1) * 128],
                        ow_sb[:, k, :], start=(k == 0), stop=(k == FG - 1))
                ac = mp.tile([128, DH], dt, name="ac", tag="ac")
                nc.vector.tensor_add(ac[:], ps[:], obb[:])
                nc.vector.tensor_scalar(
                    ac[:], ac[:], dec_sb[:, m:m + 1], None, ALU.mult)
                nc.sync.dma_start(acoef[:, m, :], ac[:])

        # ---------------- big matmuls via matmul_tile_kernel ------------
        # A: kernel spectrum  ArAi = dft.T @ acoef   (K=2048, M=4352, N=384)
        matmul_tile_kernel(tc, dft[:], acoef[:], arai[:])
        # B: uT = silu(u_wa.T @ xTa)                 (K=1152, M=384, N=8192)
        matmul_tile_kernel(tc, u_wa[:], xTa[:], uT[:], psum_evict_fn=silu_evict)
        # C: v = silu(xTa.T @ v_wa)                  (K=1152, M=8192, N=384)
        matmul_tile_kernel(tc, xTa[:], v_wa[:], v[:], psum_evict_fn=silu_evict)
        # D: forward DFT of v per batch
        for b in range(B):
            matmul_tile_kernel(
                tc, dft[:],
                v[:, b * (N // 128):(b + 1) * (N // 128), :],
                xrxi[:, b * 2 * KG:(b + 1) * 2 * KG, :])

        # E: pointwise complex multiply  P = A * X
        with (tc.tile_pool(name="pw", bufs=1) as pwp,
              tc.tile_pool(name="pw2", bufs=4) as pw2):
            ar_sb = pwp.tile([128, 2 * KG, DH], dt)
            nc.sync.dma_start(ar_sb[:], arai[:])
            for b in range(B):
                for g in range(KG):
                    xr = pw2.tile([128, DH], dt, name="xr", tag="xr")
                    xi = pw2.tile([128, DH], dt, name="xi", tag="xi")
                    nc.sync.dma_start(xr[:], xrxi[:, b * 2 * KG + g, :])
                    nc.sync.dma_start(xi[:], xrxi[:, b * 2 * KG + KG + g, :])
                    pr = pw2.tile([128, DH], dt, name="pr", tag="pr")
                    pi = pw2.tile([128, DH], dt, name="pi", tag="pi")
                    t1 = pw2.tile([128, DH], dt, name="t1", tag="t1")
                    nc.vector.tensor_mul(pr[:], ar_sb[:, g, :], xr[:])
                    nc.vector.tensor_mul(t1[:], ar_sb[:, KG + g, :], xi[:])
                    nc.vector.tensor_sub(pr[:], pr[:], t1[:])
                    nc.vector.tensor_mul(pi[:], ar_sb[:, g, :], xi[:])
                    nc.vector.tensor_mul(t1[:], ar_sb[:, KG + g, :], xr[:])
                    nc.vector.tensor_add(pi[:], pi[:], t1[:])
                    nc.sync.dma_start(prpi[:, b * 2 * KG + g, :], pr[:])
                    nc.sync.dma_start(prpi[:, b * 2 * KG + KG + g, :], pi[:])

        # F: inverse DFT  tvT_b = PrPi_b.T @ idft_cs  (K=4352, M=384, N=2048)
        for b in range(B):
            matmul_tile_kernel(
                tc, prpi[:, b * 2 * KG:(b + 1) * 2 * KG, :], idft[:],
                tvT[:, :, b * N:(b + 1) * N])

        # G: gate  gT = uT * tvT
        with tc.tile_pool(name="gate", bufs=4) as gp:
            for m in range(DH // 128):
                for nch in range(ROWS // 2048):
                    ut = gp.tile([128, 2048], dt, name="ut", tag="ut")
                    tt = gp.tile([128, 2048], dt, name="tt", tag="tt")
                    nc.sync.dma_start(ut[:], uT[:, m, nch * 2048:(nch + 1) * 2048])
                    nc.sync.dma_start(tt[:], tvT[:, m, nch * 2048:(nch + 1) * 2048])
                    nc.vector.tensor_mul(ut[:], ut[:], tt[:])
                    nc.sync.dma_start(gT[:, m, nch * 2048:(nch + 1) * 2048], ut[:])

        # H: partial o-projection  out = gT.T @ o_w  (K=384, M=8192, N=1024)
        matmul_tile_kernel(tc, gT[:], o_w[:], out[:])

    nc.compile()
    return nc


def _get_nc():
    if "nc" not in _CACHE:
        _CACHE["nc"] = _build()
    return _CACHE["nc"]


def kernel(x, u_w, u_b, v_w, v_b, o_w, o_b,
           pos_w, pos_b, lw0, lb0, lw1, lb1, lw2, lb2, out_w, out_b):
    from concourse.bass_utils import run_bass_kernel_spmd

    dft3, idft3, decay_t = _consts()
    x_flat = np.asarray(x, np.float32).reshape(ROWS, E)
    xTa = np.zeros((KA, ROWS), np.float32)
    xTa[:E] = x_flat.T
    xTa[E] = 1.0
    xTa3 = _t3(xTa)

    p_aug = np.stack([np.arange(N, dtype=np.float32),
                      np.ones(N, np.float32)])
    pw_aug = np.concatenate([pos_w, pos_b[None, :]], 0).astype(np.float32)
    # lbs layout: [:, li*4 + g] = lb_li[g*128 + p]
    lbs = np.concatenate(
        [lb.reshape(R // 128, 128).T for lb in (lb0, lb1, lb2)],
        axis=1).astype(np.float32)

    in_maps = []
    for h in range(H):
        sl = slice(h * DH, (h + 1) * DH)
        u_wa = np.zeros((KA, DH), np.float32)
        u_wa[:E] = u_w[:, sl]
        u_wa[E] = u_b[sl]
        v_wa = np.zeros((KA, DH), np.float32)
        v_wa[:E] = v_w[:, sl]
        v_wa[E] = v_b[sl]
        in_maps.append(dict(
            xTa=xTa3, u_wa=_t3(u_wa), v_wa=_t3(v_wa),
            o_w=_t3(np.ascontiguousarray(o_w[sl, :]).astype(np.float32)),
            p_aug=p_aug, pw_aug=pw_aug,
            lw0=_t3(lw0.astype(np.float32)), lw1=_t3(lw1.astype(np.float32)),
            lw2=_t3(lw2.astype(np.float32)), lbs=lbs,
            out_w=_t3(np.ascontiguousarray(out_w[:, sl]).astype(np.float32)),
            outb=np.ascontiguousarray(out_b[None, sl]).astype(np.float32),
            decay=decay_t, dft=dft3, idft=idft3,
        ))

    nc = _get_nc()
    res = run_bass_kernel_spmd(nc, in_maps, core_ids=list(range(8)),
                               trace=bool(_CACHE.get("trace")))
    _CACHE["last_res"] = res
    acc = np.zeros((ROWS, E), np.float32)
    for i in range(H):
        acc += _from3(res.results[i]["out"])
    acc += o_b[None, :]
    return acc.reshape(B, N, E)



# revision 3
# speedup vs baseline: 2.1041x; 2.1041x over previous
"""GTU (gated Toeplitz unit) Bass kernel for 8 TRN2 NeuronCores — v2.

Sharding: tensor-parallel over heads (H=8 -> 1 head/core); host sums the
8 partial o-projections.

v2 vs baseline:
- All big matmuls in bf16 (1 cycle/row on PE vs 4 for fp32); RPE MLP in
  f32r (same storage as fp32, 1 cycle/row at free-dim>=256).
- Kernel lags truncated at L=768 (decay gamma^768 ~ 4.4e-4), shrinking
  the circular conv from 4096 to M2=2816 points.
- One SBUF-resident DFT matrix per phase (loaded once, not per batch);
  forward spectra, complex multiply and gate all stay on-chip; only the
  P spectrum round-trips DRAM between the two DFT phases.
- u/v projections fused into a single pass over x^T.
"""

import numpy as np
import ml_dtypes

B, N, E = 4, 2048, 1024
H = 8
D1 = 3 * E
DH = D1 // H            # 384
R = 512
GAMMA = 0.99
EPS = 1e-8
L = 768                 # truncated kernel lags (6*128)
LC = L // 128           # 6
M2 = 2816               # circular conv length >= N + L - 1
KH = M2 // 2 + 1        # 1409 rfft bins
KC = 12                 # freq chunks of 128 (pad 1409 -> 1536)
KP = KC * 128           # 1536
ROWS = B * N            # 8192
KA = 1152               # augmented contraction for x (bias row), 9*128

_CACHE = {}

bfl = ml_dtypes.bfloat16


def _t3(a, dtype=np.float32):
    """(M, N) -> (128, M/128, N) partition-tiled layout."""
    m, n = a.shape
    assert m % 128 == 0
    return np.ascontiguousarray(
        a.reshape(m // 128, 128, n).transpose(1, 0, 2)).astype(dtype)


def _from3(a):
    p, m, n = a.shape
    return np.ascontiguousarray(
        np.asarray(a, np.float32).transpose(1, 0, 2)).reshape(m * 128, n)


def _consts():
    if "dft" in _CACHE:
        return _CACHE["dft"]
    t = np.arange(N, dtype=np.float64)[:, None]
    k = np.arange(KP, dtype=np.float64)[None, :]
    mask = (k <= (KH - 1)).astype(np.float64)
    ang = 2.0 * np.pi * t * k / M2
    cr = np.cos(ang) * mask
    ci = -np.sin(ang) * mask
    wd = np.concatenate([cr, ci], axis=1)                 # (2048, 3072)

    kk = np.arange(KP, dtype=np.float64)[:, None]
    tt = np.arange(N, dtype=np.float64)[None, :]
    w = np.where((kk == 0) | (kk == M2 // 2), 1.0, 2.0) * (kk <= (KH - 1)) / M2
    ang2 = 2.0 * np.pi * kk * tt / M2
    icos = w * np.cos(ang2)                               # (1536, 2048)
    isin = -w * np.sin(ang2)
    wf = np.concatenate([icos, isin], axis=0)             # (3072, 2048)

    decay = GAMMA ** np.arange(L, dtype=np.float64)       # lag 0 -> 1.0
    decay_t = decay.reshape(LC, 128).T                    # (128, 6)
    _CACHE["dft"] = (_t3(wd, bfl), _t3(wf, bfl), decay_t.astype(np.float32))
    return _CACHE["dft"]


def _build():
    import concourse.bass as bass
    import concourse.mybir as mybir
    import concourse.tile as tile
    from concourse import bacc

    AFT = mybir.ActivationFunctionType
    ALU = mybir.AluOpType
    f32 = mybir.dt.float32
    f32r = mybir.dt.float32r
    bf16 = mybir.dt.bfloat16

    nc = bacc.Bacc(None, target_bir_lowering=False, debug=False, num_devices=8)

    def din(name, shape, dt=f32):
        return nc.dram_tensor(name, list(shape), dt, kind="ExternalInput")

    def dint(name, shape, dt=bf16):
        return nc.dram_tensor(name, list(shape), dt)

    # inputs
    xTa = din("xTa", (128, KA // 128, ROWS), bf16)
    u_wa = din("u_wa", (128, KA // 128, DH), bf16)
    v_wa = din("v_wa", (128, KA // 128, DH), bf16)
    o_w3 = din("o_w3", (128, DH // 128, E), bf16)
    wd_d = din("wd", (128, N // 128, 2 * KP), bf16)
    wf_d = din("wf", (128, 2 * KC, N), bf16)
    p_aug = din("p_aug", (2, L))
    pw_aug = din("pw_aug", (2, R))
    lws = [din(f"lw{i}", (128, R // 128, R), bf16) for i in range(3)]
    lbs = din("lbs", (128, 3 * (R // 128)))
    out_w3 = din("out_w3", (128, R // 128, DH), bf16)
    outb = din("outb", (1, DH))
    decay = din("decay", (128, LC))
    out = nc.dram_tensor("out", [128, ROWS // 128, E], f32,
                         kind="ExternalOutput")

    # dram temps (bf16)
    uT_d = dint("uT_d", (128, DH // 128, ROWS))
    v_d = dint("v_d", (128, ROWS // 128, DH))
    psp_d = dint("psp_d", (128, B * 2 * KC, DH))

    FG = R // 128             # 4 feature groups
    NCH = L // 384            # 2 position chunks in the (truncated) MLP

    with tile.TileContext(nc) as tc, nc.allow_low_precision(
            reason="bf16 pipeline validated against fp32 reference"):
        with tc.tile_pool(name="persist", bufs=1) as pp:
            acoef = pp.tile([128, LC, DH], bf16)   # truncated decayed coefs

            # ------- RPE MLP + u/v projections (concurrent engines) -----
            # The MLP's serial norm->matmul chains leave the PE idle; the
            # u/v GEMMs stream through the same window and fill it.
            with (tc.tile_pool(name="mlp", bufs=1) as mp,
                  tc.tile_pool(name="mlp_ps", bufs=1, space="PSUM") as mps,
                  tc.tile_pool(name="uvw", bufs=1) as wp,
                  tc.tile_pool(name="uvx", bufs=3) as xp,
                  tc.tile_pool(name="uvs", bufs=4) as sp,
                  tc.tile_pool(name="uv_ps", bufs=2, space="PSUM") as ups):
                uw_sb = wp.tile([128, KA // 128, DH], bf16)
                vw_sb = wp.tile([128, KA // 128, DH], bf16)
                nc.sync.dma_start(uw_sb[:], u_wa[:])
                nc.sync.dma_start(vw_sb[:], v_wa[:])
                for grp in range(ROWS // 512):
                    xt = xp.tile([128, KA // 128, 512], bf16, name="xt",
                                 tag="xt")
                    nc.sync.dma_start(
                        xt[:], xTa[:, :, grp * 512:(grp + 1) * 512])
                    # uT tile: out[M=DH, N=512 rows]
                    for m in range(DH // 128):
                        ps = ups.tile([128, 512], f32, name="bps", tag="bps")
                        for kc in range(KA // 128):
                            nc.tensor.matmul(
                                ps[:], uw_sb[:, kc, m * 128:(m + 1) * 128],
                                xt[:, kc, :], start=(kc == 0),
                                stop=(kc == KA // 128 - 1))
                        ut = sp.tile([128, 512], bf16, name="ut", tag="ut")
                        nc.scalar.activation(ut[:], ps[:], AFT.Silu)
                        nc.sync.dma_start(
                            uT_d[:, m, grp * 512:(grp + 1) * 512], ut[:])
                    # v tiles: out[M=128 rows, N=DH]
                    for rs in range(4):
                        ps = ups.tile([128, DH], f32, name="cps", tag="cps")
                        for kc in range(KA // 128):
                            nc.tensor.matmul(
                                ps[:], xt[:, kc, rs * 128:(rs + 1) * 128],
                                vw_sb[:, kc, :], start=(kc == 0),
                                stop=(kc == KA // 128 - 1))
                        vt = sp.tile([128, DH], bf16, name="vt", tag="vt")
                        nc.scalar.activation(vt[:], ps[:], AFT.Silu)
                        nc.sync.dma_start(v_d[:, grp * 4 + rs, :], vt[:])
                ones_col = mp.tile([128, 1], bf16)     # K=128 -> M=1 reducer
                nc.vector.memset(ones_col[:], 1.0)
                one_row = mp.tile([1, 128], bf16)      # K=1 -> 128-part bcast
                nc.vector.memset(one_row[:], 1.0)
                one_rowf = mp.tile([1, 128], f32)
                nc.vector.memset(one_rowf[:], 1.0)
                c_sc = mp.tile([1, 1], f32)
                nc.vector.memset(c_sc[:], float(R ** -0.5))
                eps_sc = mp.tile([1, 1], f32)
                nc.vector.memset(eps_sc[:], EPS)

                pa_sb = mp.tile([2, L], f32)
                pw_sb = mp.tile([2, R], f32)
                lb_sb = mp.tile([128, 3 * FG], f32)
                nc.sync.dma_start(pa_sb[:], p_aug[:])
                nc.sync.dma_start(pw_sb[:], pw_aug[:])
                nc.sync.dma_start(lb_sb[:], lbs[:])

                # MLP runs only on the L kept lags; fp32 h, bf16 matmuls
                h = [mp.tile([128, L], f32, name=f"h{g}", tag=f"h{g}")
                     for g in range(FG)]
                # h0 = pos_idx @ pos_w + pos_b (K=2, fp32), feature-major
                for g in range(FG):
                    for nch in range(NCH):
                        ps = mps.tile([128, 384], f32, name="mmps", tag="mm")
                        nc.tensor.matmul(
                            ps[:], pw_sb[:, g * 128:(g + 1) * 128],
                            pa_sb[:, nch * 384:(nch + 1) * 384],
                            start=True, stop=True)
                        nc.vector.tensor_copy(
                            h[g][:, nch * 384:(nch + 1) * 384], ps[:])

                def srms_relu(h_in, phi_out):
                    # s[t] = sum_f h^2 ; factor = 1/(sqrt(s)/sqrt(R) + eps)
                    sq = [mp.tile([128, L], bf16, name=f"sq{g}", tag=f"sq{g}")
                          for g in range(FG)]
                    for g in range(FG):
                        nc.vector.tensor_mul(sq[g][:], h_in[g][:], h_in[g][:])
                    facb = mp.tile([1, L], bf16, name="facb", tag="facb")
                    fac = mp.tile([1, L], f32, name="fac", tag="fac")
                    for nch in range(NCH):
                        ps1 = mps.tile([1, 384], f32, name="redps", tag="red")
                        for g in range(FG):
                            nc.tensor.matmul(
                                ps1[:], ones_col[:],
                                sq[g][:, nch * 384:(nch + 1) * 384],
                                start=(g == 0), stop=(g == FG - 1))
                        sl = fac[:, nch * 384:(nch + 1) * 384]
                        nc.scalar.activation(sl, ps1[:], AFT.Sqrt)
                        nc.vector.tensor_scalar(
                            sl, sl, c_sc[:], eps_sc[:], ALU.mult, ALU.add)
                        nc.vector.reciprocal(
                            facb[:, nch * 384:(nch + 1) * 384], sl)
                    fb = mp.tile([128, L], f32, name="fb", tag="fb")
                    for nch in range(NCH):
                        psb = mps.tile([128, 384], f32, name="bcps", tag="bc")
                        nc.tensor.matmul(
                            psb[:], one_row[:],
                            facb[:, nch * 384:(nch + 1) * 384],
                            start=True, stop=True)
                        nc.vector.tensor_copy(
                            fb[:, nch * 384:(nch + 1) * 384], psb[:])
                    for g in range(FG):
                        nc.vector.tensor_mul(phi_out[g][:], h_in[g][:], fb[:])
                        nc.scalar.activation(
                            phi_out[g][:], phi_out[g][:], AFT.Relu)

                phi = [mp.tile([128, L], bf16, name=f"phi{g}", tag=f"phi{g}")
                       for g in range(FG)]
                srms_relu(h, phi)

                lw_sb = mp.tile([128, FG, R], bf16)
                for li in range(3):
                    nc.sync.dma_start(lw_sb[:], lws[li][:])
                    for g in range(FG):
                        for nch in range(NCH):
                            ps = mps.tile([128, 384], f32, name="mmps",
                                          tag="mm")
                            for kk in range(FG):
                                nc.tensor.matmul(
                                    ps[:],
                                    lw_sb[:, kk, g * 128:(g + 1) * 128],
                                    phi[kk][:, nch * 384:(nch + 1) * 384],
                                    start=(kk == 0), stop=(kk == FG - 1))
                            sl = h[g][:, nch * 384:(nch + 1) * 384]
                            nc.vector.tensor_scalar(
                                sl, ps[:],
                                lb_sb[:, li * FG + g:li * FG + g + 1],
                                None, ALU.add)
                    srms_relu(h, phi)

                # coefs (t-major, lags < L only) = phi.T @ out_w
                ow_sb = mp.tile([128, FG, DH], bf16)
                ob_sb = mp.tile([1, DH], f32)
                dec_sb = mp.tile([128, LC], f32)
                nc.sync.dma_start(ow_sb[:], out_w3[:])
                nc.sync.dma_start(ob_sb[:], outb[:])
                nc.sync.dma_start(dec_sb[:], decay[:])
                obb = mp.tile([128, DH], f32)
                psb = mps.tile([128, DH], f32, name="bc2ps", tag="bc")
                nc.tensor.matmul(psb[:], one_rowf[:], ob_sb[:],
                                 start=True, stop=True)
                nc.vector.tensor_copy(obb[:], psb[:])
                for m in range(LC):
                    ps = mps.tile([128, DH], f32, name="mm2ps", tag="mm")
                    for kk in range(FG):
                        nc.tensor.matmul(
                            ps[:], phi[kk][:, m * 128:(m + 1) * 128],
                            ow_sb[:, kk, :], start=(kk == 0),
                            stop=(kk == FG - 1))
                    ac = mp.tile([128, DH], f32, name="ac", tag="ac")
                    nc.vector.tensor_add(ac[:], ps[:], obb[:])
                    nc.vector.tensor_scalar(
                        acoef[:, m, :], ac[:], dec_sb[:, m:m + 1],
                        None, ALU.mult)

            # ---------------- forward DFTs + complex multiply -----------
            # m-tile KC+11 (sin rows at the Nyquist chunk) is identically
            # zero: sin(pi*t) = 0. Skip it in A/D and its product in F;
            # at j=11 only the real product survives.
            with (tc.tile_pool(name="wd", bufs=1) as wdp,
                  tc.tile_pool(name="fwd", bufs=1) as fp_,
                  tc.tile_pool(name="fwdv", bufs=2) as fpv,
                  tc.tile_pool(name="fwd2", bufs=4) as fp2,
                  tc.tile_pool(name="fwd_ps", bufs=4, space="PSUM") as fps):
                wd_sb = wdp.tile([128, N // 128, 2 * KP], bf16)
                # split load: A only contracts the first LC row-chunks
                nc.sync.dma_start(wd_sb[:, :LC, :], wd_d[:, :LC, :])
                nc.sync.dma_start(wd_sb[:, LC:, :], wd_d[:, LC:, :])
                asp = fp_.tile([128, 2 * KC, DH], bf16)    # kernel spectrum
                # A: Ar/Ai m-tiles, contraction over L lags only
                for m in range(2 * KC - 1):
                    ps = fps.tile([128, DH], f32, name="aps", tag="aps")
                    for kc in range(LC):
                        nc.tensor.matmul(
                            ps[:], wd_sb[:, kc, m * 128:(m + 1) * 128],
                            acoef[:, kc, :], start=(kc == 0),
                            stop=(kc == LC - 1))
                    nc.scalar.activation(asp[:, m, :], ps[:], AFT.Copy)
                xsp = fp_.tile([128, 2 * KC, DH], bf16)
                for b in range(B):
                    vb = fpv.tile([128, N // 128, DH], bf16, name="vb",
                                  tag="vb")
                    nc.sync.dma_start(
                        vb[:], v_d[:, b * (N // 128):(b + 1) * (N // 128), :])
                    # D: X = DFT(v_b); interleave re/im pairs for E
                    for j in range(KC):
                        ms = (j,) if j == KC - 1 else (j, KC + j)
                        for m in ms:
                            ps = fps.tile([128, DH], f32, name="dps",
                                          tag="dps")
                            for kc in range(N // 128):
                                nc.tensor.matmul(
                                    ps[:],
                                    wd_sb[:, kc, m * 128:(m + 1) * 128],
                                    vb[:, kc, :], start=(kc == 0),
                                    stop=(kc == N // 128 - 1))
                            nc.scalar.activation(xsp[:, m, :], ps[:],
                                                 AFT.Copy)
                        # E: P = A * X (complex), in place over xsp
                        xr, xi = xsp[:, j, :], xsp[:, KC + j, :]
                        ar, ai = asp[:, j, :], asp[:, KC + j, :]
                        if j == KC - 1:
                            nc.vector.tensor_mul(xr, ar, xr)
                            continue
                        t1 = fp2.tile([128, DH], bf16, name="t1", tag="t1")
                        t2 = fp2.tile([128, DH], bf16, name="t2", tag="t2")
                        t3 = fp2.tile([128, DH], bf16, name="t3", tag="t3")
                        nc.vector.tensor_mul(t1[:], ar, xr)
                        nc.vector.tensor_mul(t2[:], ai, xi)
                        nc.vector.tensor_mul(t3[:], ar, xi)
                        nc.vector.tensor_mul(xi, ai, xr)
                        nc.vector.tensor_sub(xr, t1[:], t2[:])
                        nc.vector.tensor_add(xi, t3[:], xi)
                    nc.sync.dma_start(
                        psp_d[:, b * 2 * KC:(b + 1) * 2 * KC - 1, :],
                        xsp[:, :2 * KC - 1, :])

            # ---------------- inverse DFT + gate + o-projection ---------
            with (tc.tile_pool(name="wf", bufs=1) as wfp,
                  tc.tile_pool(name="inv", bufs=2) as ip_,
                  tc.tile_pool(name="invg", bufs=2) as gp_,
                  tc.tile_pool(name="invw", bufs=1) as owp,
                  tc.tile_pool(name="invs", bufs=4) as isp,
                  tc.tile_pool(name="inv_ps", bufs=4, space="PSUM") as ips):
                NJ = 2 * KC - 1        # Nyquist-sin chunk dropped
                wf_sb = wfp.tile([128, NJ, N], bf16)
                nc.sync.dma_start(wf_sb[:], wf_d[:, :NJ, :])
                ow_sb = owp.tile([128, DH // 128, E], bf16)
                nc.sync.dma_start(ow_sb[:], o_w3[:])
                for b in range(B):
                    pb = ip_.tile([128, NJ, DH], bf16, name="pb", tag="pb")
                    ub = ip_.tile([128, DH // 128, N], bf16, name="ub",
                                  tag="ub")
                    nc.sync.dma_start(
                        pb[:], psp_d[:, b * 2 * KC:b * 2 * KC + NJ, :])
                    nc.sync.dma_start(
                        ub[:], uT_d[:, :, b * N:(b + 1) * N])
                    gb = gp_.tile([128, DH // 128, N], bf16, name="gb",
                                  tag="gb")
                    # F: tv^T = sum_k P * WF ; gate with u in the evict
                    for m in range(DH // 128):
                        for tch in range(N // 512):
                            ps = ips.tile([128, 512], f32, name="fps",
                                          tag="fps")
                            for j in range(NJ):
                                nc.tensor.matmul(
                                    ps[:], pb[:, j, m * 128:(m + 1) * 128],
                                    wf_sb[:, j, tch * 512:(tch + 1) * 512],
                                    start=(j == 0), stop=(j == NJ - 1))
                            sl = slice(tch * 512, (tch + 1) * 512)
                            nc.vector.tensor_mul(
                                gb[:, m, sl], ps[:], ub[:, m, sl])
                    # H: partial o-projection out_b = g^T @ o_w
                    for mo in range(N // 128):
                        for ech in range(E // 512):
                            ps = ips.tile([128, 512], f32, name="hps",
                                          tag="hps")
                            for kc in range(DH // 128):
                                nc.tensor.matmul(
                                    ps[:], gb[:, kc, mo * 128:(mo + 1) * 128],
                                    ow_sb[:, kc, ech * 512:(ech + 1) * 512],
                                    start=(kc == 0), stop=(kc == DH // 128 - 1))
                            ot = isp.tile([128, 512], f32, name="ot",
                                          tag="ot")
                            nc.scalar.activation(ot[:], ps[:], AFT.Copy)
                            nc.sync.dma_start(
                                out[:, b * (N // 128) + mo,
                                    ech * 512:(ech + 1) * 512], ot[:])

    nc.compile()
    return nc


def _get_nc():
    if "nc" not in _CACHE:
        _CACHE["nc"] = _build()
    return _CACHE["nc"]


def kernel(x, u_w, u_b, v_w, v_b, o_w, o_b,
           pos_w, pos_b, lw0, lb0, lw1, lb1, lw2, lb2, out_w, out_b):
    from concourse.bass_utils import run_bass_kernel_spmd

    wd3, wf3, decay_t = _consts()
    x_flat = np.asarray(x, np.float32).reshape(ROWS, E)
    xTa = np.zeros((KA, ROWS), np.float32)
    xTa[:E] = x_flat.T
    xTa[E] = 1.0
    xTa3 = _t3(xTa, bfl)

    p_aug = np.stack([np.arange(L, dtype=np.float32),
                      np.ones(L, np.float32)])
    pw_aug = np.concatenate([pos_w, pos_b[None, :]], 0).astype(np.float32)
    lbs = np.concatenate(
        [bb.reshape(R // 128, 128).T for bb in (lb0, lb1, lb2)],
        axis=1).astype(np.float32)

    in_maps = []
    for h in range(H):
        sl = slice(h * DH, (h + 1) * DH)
        u_wa = np.zeros((KA, DH), np.float32)
        u_wa[:E] = u_w[:, sl]
        u_wa[E] = u_b[sl]
        v_wa = np.zeros((KA, DH), np.float32)
        v_wa[:E] = v_w[:, sl]
        v_wa[E] = v_b[sl]
        in_maps.append(dict(
            xTa=xTa3, u_wa=_t3(u_wa, bfl), v_wa=_t3(v_wa, bfl),
            o_w3=_t3(np.ascontiguousarray(o_w[sl, :]).astype(np.float32), bfl),
            wd=wd3, wf=wf3,
            p_aug=p_aug, pw_aug=pw_aug,
            lw0=_t3(lw0, bfl), lw1=_t3(lw1, bfl), lw2=_t3(lw2, bfl), lbs=lbs,
            out_w3=_t3(np.ascontiguousarray(out_w[:, sl]), bfl),
            outb=np.ascontiguousarray(out_b[None, sl]).astype(np.float32),
            decay=decay_t,
        ))

    nc = _get_nc()
    res = run_bass_kernel_spmd(nc, in_maps, core_ids=list(range(8)),
                               trace=bool(_CACHE.get("trace")))
    _CACHE["last_res"] = res
    acc = np.zeros((ROWS, E), np.float32)
    for i in range(H):
        acc += _from3(res.results[i]["out"])
    acc += o_b[None, :]
    return acc.reshape(B, N, E)
